# revision 75
# baseline (speedup 1.0000x reference)
"""Trainium2 Bass kernel for nn_C2BM_30537217474758 (gnn_message_passing).

Concept-bottleneck model:
  x_enc = lrelu(x @ W_enc + b_enc)                         [B, 1024]
  vals  = lrelu(einsum('bi,rio->bro', x_enc, Wv) + bv)     [B, 8, 256]
  p_root = softmax(einsum('bro,roc->brc', vals, Ws) + bs)  [B, 8, 4]
  p_root = intervene(p_root, c[:, :8], ii[:, :8])
  h     = lrelu(einsum('bp,nph->bnh', p_root.flat, W1c) + b1c)
  p_mid = softmax(einsum('bnh,nhc->bnc', h, W2c) + b2c); intervene
  y     = softmax(lrelu(p_mid.flat @ W1y + b1y) @ W2y + b2y)
  out   = concat([p_root, p_mid, y[:, None]], axis=1)      [B, 17, 4]

Strategy: pure data-parallel over 8 NeuronCores (batch shard 1024/core),
weights replicated. Channels live on SBUF partitions and batch on the free
dimension for the two large GEMMs (contraction on partitions); x is
transposed on-chip via the PE (bf16 cast during the SWDGE DMA load, fp8
cast on the PSUM drain). The two large GEMMs and the root scorer run in
fp8(e4m3) DoubleRow mode (2x PE throughput, contraction 256/instr) with
weights pre-scaled by 64 on the host and rescaled at the fp32 PSUM drain;
softmax / intervention arithmetic is fp32; the tiny mid/task propagators
stay bf16. The batch is processed in two 512-row halves so each half's
softmax->propagator->task tail (DVE/ACT latency chains) hides under the
other half's GEMMs. Small biases are folded into the matmuls via ones-row
augmented operands.

Hard-won scheduling facts baked in here (measured on hw):
- All large loads ride ONE SWDGE queue in exact consumption order
  (x bt0-3, wenc, wv, x bt4-7): the SWDGE packet stream dominates HBM
  arbitration and any concurrent HWDGE queue starves to ~70 GB/s, so
  queue order is the only reliable priority mechanism. Tiny tensors ride
  the SP HWDGE ring; nothing rides the ACT ring (its DMA enqueues would
  compete with PSUM drains on the ACT sequencer).
- Prelu (parametric_relu) is used instead of Lrelu: identical math, but
  it shares the "exp_and_others" activation table-set with Exp, so the
  ACT engine never reloads tables (~1.3-2.7us each) between GEMM drains
  and the softmax chains.
- Intervention one-hots/masks are precomputed host-side and DMA'd in the
  exact device layout: any on-device op that depends on a late DMA can be
  hoisted by the scheduler ahead of the transpose drains on DVE and
  head-of-line block the whole PE pipeline.
- fp8 DoubleRow 512-col matmuls issue back-to-back at ~216 ns (1 moving
  col/cycle, 2 contraction rows/cycle = 157 TF/s); dummy warm-up matmuls
  at t~9.5us ramp the PE p-state before the first transposes, and small
  warm bursts in the final tail keep it from downclocking between
  latency-bound softmax stages.
"""

import os
import sys

try:
    import concourse  # noqa: F401
except ImportError:
    sys.path.insert(0, "/opt/trn_rl_repo")

import numpy as np
import ml_dtypes

import concourse.bacc as bacc
import concourse.tile as tile
from concourse import mybir

# ---------------- problem constants (hardcoded per contract) ----------------
B, D_IN, D_H = 8192, 2048, 1024
N_ROOT, N_MID, CARD, CHS = 8, 8, 4, 64
OV = CARD * CHS           # 256  value-embedding width per root
P_IN = N_ROOT * CARD      # 32
P_HID = 2 * P_IN          # 64
N_CORES = 8
BSH = B // N_CORES        # 1024 batch rows per core
NBT = BSH // 128          # 8 partition-tiles of batch
KT_IN = D_IN // 128       # 16 contraction tiles for encoder
KT_H = D_H // 128         # 8 contraction tiles for Wv
OUTW = 17 * CARD          # 68 output cols per row
WSCALE = 64.0             # host pre-scale on fp8 weights

F32 = mybir.dt.float32
I32 = mybir.dt.int32
BF16 = mybir.dt.bfloat16
FP8 = mybir.dt.float8e4
AF = mybir.ActivationFunctionType
ALU = mybir.AluOpType
AX = mybir.AxisListType
DR = mybir.MatmulPerfMode.DoubleRow

LRELU_ALPHA = 0.01
# CoreSim does not implement Lrelu/Prelu; BASS_SIM_SAFE=1 swaps in Relu so
# the rest of the program can be validated in simulation.
SIM_SAFE = os.environ.get("BASS_SIM_SAFE") == "1"
# BASS_NO_DR=1 replaces each DoubleRow matmul with two plain matmuls
# (identical tiles/layout) in case the simulator chokes on DR fp8.
NO_DR = os.environ.get("BASS_NO_DR") == "1"
# Prelu (parametric_relu) is numerically identical to Lrelu but lives in
# the SAME activation table-set as Exp ("exp_and_others"), so the ACT
# engine never reloads tables between GEMM drains and softmax exps
# (each reload is ~1.3-2.7us, and they land on the critical tail).
ACT_LRELU = AF.Relu if SIM_SAFE else AF.Prelu


def build_program(zero_bias=False):
    """Emit the per-core Bass program (identical on all 8 cores).

    zero_bias=True specializes for all-zero b_enc/bv (checked at runtime
    in kernel()): the encoder / value-GEMM PSUM drains then need no
    per-partition bias AP and can drain two banks per ACT instruction.
    """
    nc = bacc.Bacc("TRN2", target_bir_lowering=False, debug=False,
                   num_devices=N_CORES)

    def mm_dr(out, lhsT2, rhs2, start, stop, skip_group_check=False):
        """DoubleRow fp8 matmul; lhsT2/rhs2 are [128, 2, F] pair views."""
        if NO_DR:
            for i in range(2):
                nc.tensor.matmul(out, lhsT2[:, i, :], rhs2[:, i, :],
                                 start=(start and i == 0),
                                 stop=(stop and i == 1),
                                 skip_group_check=skip_group_check)
        else:
            nc.tensor.matmul(out, lhsT2, rhs2, start=start, stop=stop,
                             perf_mode=DR, skip_group_check=skip_group_check)

    # ------------- DRAM I/O -------------
    x_d = nc.dram_tensor("x", [BSH, D_IN], F32, kind="ExternalInput")
    # intervention one-hots/masks are precomputed on the host in the device
    # layout [p, (g,lv), bti, grp, card] (pure contiguous DMA, no on-device
    # label arithmetic needed)
    oh_d = nc.dram_tensor("oh", [128, 4 * 128], F32, kind="ExternalInput")
    m_d = nc.dram_tensor("m", [128, 4 * 128], I32, kind="ExternalInput")
    wenc_d = nc.dram_tensor("wenc", [D_IN, D_H], FP8, kind="ExternalInput")
    wv_d = nc.dram_tensor("wv", [D_H, N_ROOT * OV], FP8, kind="ExternalInput")
    ws_d = nc.dram_tensor("ws", [OV, N_ROOT * CARD], FP8, kind="ExternalInput")
    # [33, 4, 128]: rows 0-31 W1c mid-pairs, row 32 = b1c (ones-row fold)
    w1c_d = nc.dram_tensor("w1c", [P_IN + 1, 4 * 128], BF16, kind="ExternalInput")
    w2c_d = nc.dram_tensor("w2c", [128, 4 * 8], BF16, kind="ExternalInput")
    w1y_d = nc.dram_tensor("w1y", [P_IN + 1, P_HID], BF16, kind="ExternalInput")
    w2y_d = nc.dram_tensor("w2y", [P_HID + 1, CARD], BF16, kind="ExternalInput")
    benc_d = nc.dram_tensor("benc", [128, KT_H], F32, kind="ExternalInput")
    bv_d = nc.dram_tensor("bv", [128, 16], F32, kind="ExternalInput")
    # bias rows pre-tiled 4x so one 128-col matmul seeds a whole logits tile
    bsr_d = nc.dram_tensor("bsr", [1, 128], BF16, kind="ExternalInput")
    b2cr_d = nc.dram_tensor("b2cr", [1, 128], BF16, kind="ExternalInput")
    ident_d = nc.dram_tensor("ident", [128, 128], BF16, kind="ExternalInput")
    out_d = nc.dram_tensor("out", [BSH, OUTW], F32, kind="ExternalOutput")

    with tile.TileContext(nc) as tc:
        with (
            tc.tile_pool(name="persist", bufs=1) as persist,
            tc.tile_pool(name="xraw", bufs=8) as xraw_pool,
            tc.tile_pool(name="vals", bufs=3) as vals_pool,
            tc.tile_pool(name="stage", bufs=3) as stage_pool,
            tc.tile_pool(name="tmp32", bufs=4) as tmp32_pool,
            tc.tile_pool(name="tmp8", bufs=6) as tmp8_pool,
            tc.tile_pool(name="outp", bufs=2) as out_pool,
            tc.tile_pool(name="ps_mm", bufs=4, space="PSUM") as ps_mm,
            tc.tile_pool(name="ps_lg", bufs=2, space="PSUM") as ps_lg,
            tc.tile_pool(name="ps_tr", bufs=2, space="PSUM") as ps_tr,
        ):
            # PE p-state warm-up: dummy matmuls on a memset tile ramp the
            # tensor engine to full clock while the first x tile is in
            # flight (cold-start matmuls otherwise run ~40% slower)
            wdum = persist.tile([128, 128], BF16)
            nc.vector.memset(wdum, 0.0)
            if zero_bias:
                pswu = ps_mm.tile([128, 1024], F32, tag="mm", bufs=2,
                                  name="warmup")
            else:
                pswu = ps_mm.tile([128, 512], F32, tag="mm", name="warmup")
            for _ in range(36):
                nc.tensor.matmul(pswu[:, 0:128], wdum, wdum,
                                 start=True, stop=True)

            # -------- x pipeline: the SWDGE packet stream starves concurrent
            # HWDGE queues, so only bt0-3 (needed first, casting f32->bf16)
            # go through SWDGE. bt4-7 ride the SP HWDGE queue as fp32
            # *behind* the weights (see below) and are cast to bf16 on the
            # otherwise idle GpSimd vector engine.
            # ALL large transfers ride the single SWDGE queue in exact
            # consumption order: x bt0-3, wenc, wv, x bt4-7. The SWDGE
            # packet stream dominates HBM arbitration, so concurrent HWDGE
            # queues starve — one totally-ordered stream is the only
            # reliable way to enforce load priority.
            # wenc chunks interleave between the early x tiles: SWDGE
            # completion semaphores post with multi-us lag when the queue is
            # deep, so the encoder's weights must be IN FLIGHT before bt3,
            # not after. wenc is two separate tiles so the encoder's
            # partial-k start depends only on its own chunk.
            xbs = []

            def load_x(bt):
                src = x_d.ap()[bt * 128:(bt + 1) * 128, :]
                xb = xraw_pool.tile([128, D_IN], BF16, tag="xb", bufs=8)
                nc.gpsimd.dma_start(out=xb, in_=src)  # SWDGE casts f32->bf16
                xbs.append(xb)

            wenc_r = wenc_d.ap().rearrange("(kt p) h -> p kt h", p=128)
            wenc_lo = persist.tile([128, 8, D_H], FP8)
            wenc_hi = persist.tile([128, 8, D_H], FP8)

            def wenc_sl(kt2lo, kt2hi, hsl):
                """[128, 2, 128] DR pair view of wenc kt pair (2j, 2j+1)."""
                t = wenc_lo if kt2hi <= 8 else wenc_hi
                off = 0 if kt2hi <= 8 else 8
                return t[:, kt2lo - off:kt2hi - off, hsl]

            # bt0 loads as two half-column chunks so its first transposes
            # start as soon as cols 0:1024 land (~1.3us earlier)
            xb0 = xraw_pool.tile([128, D_IN], BF16, tag="xb", bufs=8)
            nc.gpsimd.dma_start(out=xb0[:, 0:1024],
                                in_=x_d.ap()[0:128, 0:1024])
            nc.gpsimd.dma_start(out=xb0[:, 1024:2048],
                                in_=x_d.ap()[0:128, 1024:2048])
            xbs.append(xb0)
            load_x(1)
            nc.gpsimd.dma_start(out=wenc_lo, in_=wenc_r[:, 0:8, :])
            load_x(2)
            nc.gpsimd.dma_start(out=wenc_hi, in_=wenc_r[:, 8:16, :])
            load_x(3)
            # x4 ahead of wv: its transposes fill the PE's wait for the
            # second wenc chunk's completion semaphore
            load_x(4)
            wv_sb = persist.tile([128, KT_H, N_ROOT * OV], FP8)
            wv_r = wv_d.ap().rearrange("(kt p) o -> p kt o", p=128)
            for h in range(2):
                nc.gpsimd.dma_start(out=wv_sb[:, 4 * h:4 * h + 4, :],
                                    in_=wv_r[:, 4 * h:4 * h + 4, :])
            for bt in range(5, NBT):
                load_x(bt)

            # small tensors ride the otherwise-idle SP HWDGE ring (its
            # enqueues run on the SP sequencer, keeping the ACT engine free
            # for PSUM drains; transfers trickle under the SWDGE stream but
            # everything here is tiny and needed late, except ident which is
            # first and lands in ~1us even when starved)
            ident_sb = persist.tile([128, 128], BF16)
            nc.sync.dma_start(out=ident_sb, in_=ident_d.ap())
            ws_sb = persist.tile([128, 2, 32], FP8)
            nc.sync.dma_start(out=ws_sb,
                              in_=ws_d.ap().rearrange("(kt p) c -> p kt c", p=128))
            w1c_sb = persist.tile([P_IN + 1, 4, 128], BF16)
            nc.sync.dma_start(out=w1c_sb,
                              in_=w1c_d.ap().rearrange("p (q m) -> p q m", m=128))
            w2c_sb = persist.tile([128, 4, 8], BF16)
            nc.sync.dma_start(out=w2c_sb,
                              in_=w2c_d.ap().rearrange("p (q c) -> p q c", c=8))
            w1y_sb = persist.tile([P_IN + 1, P_HID], BF16)
            nc.sync.dma_start(out=w1y_sb, in_=w1y_d.ap())
            w2y_sb = persist.tile([P_HID + 1, CARD], BF16)
            nc.sync.dma_start(out=w2y_sb, in_=w2y_d.ap())
            benc_sb = persist.tile([128, KT_H], F32)
            nc.sync.dma_start(out=benc_sb, in_=benc_d.ap())
            bv_sb = persist.tile([128, 16], F32)
            nc.sync.dma_start(out=bv_sb, in_=bv_d.ap())
            bsr_sb = persist.tile([1, 128], BF16)
            nc.sync.dma_start(out=bsr_sb, in_=bsr_d.ap())
            b2cr_sb = persist.tile([1, 128], BF16)
            nc.sync.dma_start(out=b2cr_sb, in_=b2cr_d.ap())
            # oh/m (0.5 MB, not needed before ~50us) go last
            oh_sb = persist.tile([128, 4, 128], F32)
            nc.sync.dma_start(out=oh_sb,
                              in_=oh_d.ap().rearrange("p (s k) -> p s k", k=128))
            m_sb = persist.tile([128, 4, 128], I32)
            nc.sync.dma_start(out=m_sb,
                              in_=m_d.ap().rearrange("p (s k) -> p s k", k=128))
            ones_sb = persist.tile([1, 128], BF16)
            nc.vector.memset(ones_sb, 1.0)

            # xT in fp8: PE transposes the bf16 batch-tiles, DVE drains the
            # bf16 PSUM with an fp8 cast.
            xt_sb = persist.tile([128, KT_IN, BSH], FP8)  # xT: [d, b]

            def transpose_bt(bt):
                # 4 transposes share one PSUM tile; a single (strided) DVE
                # copy drains all four, quartering the drain instruction count
                xb = xbs[bt]
                for kq in range(KT_IN // 4):
                    trp = ps_tr.tile([128, 4, 128], BF16, tag="ptr")
                    for ki in range(4):
                        kt = 4 * kq + ki
                        nc.tensor.transpose(trp[:, ki, :],
                                            xb[:, kt * 128:(kt + 1) * 128],
                                            ident_sb)
                    nc.vector.tensor_copy(
                        xt_sb[:, 4 * kq:4 * kq + 4,
                              bt * 128:(bt + 1) * 128], trp)

            # ---------------- persistent activations ----------------
            xenc_sb = persist.tile([128, KT_H, BSH], FP8)   # x_encT: [h, b]
            prT_sb = persist.tile([P_HID, BSH], BF16)  # [32 p | ones] x b
            pmT_sb = persist.tile([P_HID, BSH], BF16)
            hyT_sb = persist.tile([P_HID + 1, BSH], BF16)    # row 64 = ones
            nc.vector.memset(hyT_sb[P_HID:P_HID + 1, :], 1.0)
            hT_sb = persist.tile([128, 4, BSH], BF16)  # [2 mids x 64h, b]

            # output rows for batch-tiles 4g..4g+3, packed [128, 4*68]
            osb_gs = [out_pool.tile([128, 4 * OUTW], F32, tag="osbg",
                                    name=f"osbg{i}") for i in range(2)]

            def osb_view(g, lo, hi):
                """[128, 4, hi-lo, 4] view of output cols [lo*4, hi*4)."""
                return (osb_gs[g].rearrange("p (b k) -> p b k", k=OUTW)
                        [:, :, lo * 4:hi * 4]
                        .rearrange("p b (g c) -> p b g c", c=CARD))

            # ------ host-precomputed intervention one-hots and masks --------
            # compact [128, 4bt*32] layout -> [128, 4, 8, 4] views
            oh_t = {(g, lv): oh_sb[:, 2 * g + lv, :]
                    .rearrange("p (b k) -> p b k", b=4)
                    for g in range(2) for lv in range(2)}
            m_t = {(g, lv): m_sb[:, 2 * g + lv, :]
                   .rearrange("p (b k) -> p b k", b=4)
                   for g in range(2) for lv in range(2)}

            def pview(t, bn=4):
                """[128, bn, 8, 4] view of the 32 data cols of each 64-col
                bt-block in a [128, bn*64] staging tile."""
                return (t.rearrange("p (b k) -> p b k", k=P_HID)[:, :, 0:32]
                        .rearrange("p b (g c) -> p b g c", c=CARD))

            # ---------------- encoder GEMM -> x_encT (fp8 DoubleRow) --------
            def encoder_half(bh, mid_pe=None):
                if zero_bias:
                    # two ht banks share one PSUM tile and one merged drain.
                    # The kt loop is split at the wenc DMA chunk boundary
                    # (kt0-7 land ~2.5us before kt8-15): the first two tiles
                    # run their kt0-7 partial sums as soon as chunk 1 lands,
                    # with the accumulation groups left open until chunk 2.
                    def jrange(ps, hp, j0, j1):
                        for sub in range(2):
                            for j in range(j0, j1):
                                mm_dr(ps[:, sub * 512:(sub + 1) * 512],
                                      wenc_sl(2 * j, 2 * j + 2,
                                              slice((2 * hp + sub) * 128,
                                                    (2 * hp + sub + 1) * 128)),
                                      xt_sb[:, 2 * j:2 * j + 2,
                                            bh * 512:(bh + 1) * 512],
                                      start=(j == 0),
                                      stop=(j == KT_IN // 2 - 1))

                    def drain(ps, hp):
                        nc.scalar.activation(
                            xenc_sb[:, 2 * hp:2 * hp + 2,
                                    bh * 512:(bh + 1) * 512], ps,
                            ACT_LRELU, scale=1.0 / WSCALE, alpha=LRELU_ALPHA)

                    ps0 = ps_mm.tile([128, 1024], F32, tag="mm", bufs=2)
                    ps1 = ps_mm.tile([128, 1024], F32, tag="mm", bufs=2)
                    jrange(ps0, 0, 0, 4)
                    jrange(ps1, 1, 0, 4)
                    if mid_pe:
                        mid_pe()
                    jrange(ps0, 0, 4, 8)
                    drain(ps0, 0)
                    jrange(ps1, 1, 4, 8)
                    drain(ps1, 1)
                    for hp in (2, 3):
                        ps = ps_mm.tile([128, 1024], F32, tag="mm", bufs=2)
                        jrange(ps, hp, 0, 8)
                        drain(ps, hp)
                    return
                for ht in range(KT_H):
                    ps = ps_mm.tile([128, 512], F32, tag="mm")
                    for j in range(KT_IN // 2):
                        mm_dr(ps,
                              wenc_sl(2 * j, 2 * j + 2,
                                      slice(ht * 128, (ht + 1) * 128)),
                              xt_sb[:, 2 * j:2 * j + 2,
                                    bh * 512:(bh + 1) * 512],
                              start=(j == 0), stop=(j == KT_IN // 2 - 1))
                    nc.scalar.activation(
                        xenc_sb[:, ht, bh * 512:(bh + 1) * 512], ps,
                        ACT_LRELU, bias=benc_sb[:, ht:ht + 1],
                        scale=1.0 / WSCALE, alpha=LRELU_ALPHA)

            # ------------- per-root value GEMM + scorer (one half) ----------
            def vals_scorer_half(g, lg, extra_pe=None):
                """Value embeddings + root scorer for batch rows
                [512g, 512(g+1)); 64x-scaled logits into lg [128, 4bt x 32].
                extra_pe: dict {r: fn} emitting extra PE work after root r."""
                nc.tensor.matmul(lg, ones_sb, bsr_sb, start=True, stop=False,
                                 skip_group_check=True)
                for r in range(N_ROOT):
                    vals_sb = vals_pool.tile([128, 2, 512], FP8, tag="vals")
                    if zero_bias:
                        ps = ps_mm.tile([128, 1024], F32, tag="mm", bufs=2)
                        for ot in range(2):
                            for j in range(KT_H // 2):
                                mm_dr(ps[:, ot * 512:(ot + 1) * 512],
                                      wv_sb[:, 2 * j:2 * j + 2,
                                            r * OV + ot * 128:
                                            r * OV + (ot + 1) * 128],
                                      xenc_sb[:, 2 * j:2 * j + 2,
                                              g * 512:(g + 1) * 512],
                                      start=(j == 0),
                                      stop=(j == KT_H // 2 - 1))
                        nc.scalar.activation(
                            vals_sb.rearrange("p a b -> p (a b)"), ps,
                            ACT_LRELU, scale=1.0 / WSCALE, alpha=LRELU_ALPHA)
                    else:
                        for ot in range(2):
                            ps = ps_mm.tile([128, 512], F32, tag="mm")
                            for j in range(KT_H // 2):
                                mm_dr(ps,
                                      wv_sb[:, 2 * j:2 * j + 2,
                                            r * OV + ot * 128:
                                            r * OV + (ot + 1) * 128],
                                      xenc_sb[:, 2 * j:2 * j + 2,
                                              g * 512:(g + 1) * 512],
                                      start=(j == 0),
                                      stop=(j == KT_H // 2 - 1))
                            nc.scalar.activation(
                                vals_sb[:, ot, :], ps, ACT_LRELU,
                                bias=bv_sb[:, 2 * r + ot:2 * r + ot + 1],
                                scale=1.0 / WSCALE, alpha=LRELU_ALPHA)
                    for bti in range(4):
                        dst = lg[:, bti * 32 + r * 4:bti * 32 + r * 4 + 4]
                        mm_dr(dst,
                              vals_sb[:, :, bti * 128:(bti + 1) * 128],
                              ws_sb[:, :, r * 4:(r + 1) * 4],
                              start=False, stop=True, skip_group_check=True)
                    if extra_pe and r in extra_pe:
                        extra_pe[r]()

            # ---------------- tail stages for one half ----------------
            # btlo/btn select a bt-subset so the final half's chain can be
            # pipelined in two 2-bt chunks (chunk B's PE transposes overlap
            # chunk C's softmax DVE/ACT ops)
            def softmax_chain(g, lg, lv, scale, btlo=0, btn=4):
                """exp/softmax + intervention on [128, btn x 32] logits
                (pre-scaled by 1/scale at the exp); probs -> pfin."""
                e = tmp32_pool.tile([128, 32 * btn], F32, tag="e")
                nc.scalar.activation(
                    e, lg[:, btlo * 32:(btlo + btn) * 32], AF.Exp, scale=scale)
                s = tmp8_pool.tile([128, 8 * btn], F32, tag="s")
                nc.vector.reduce_sum(s, e.rearrange("p (x c) -> p x c", c=CARD),
                                     axis=AX.X)
                rcp = tmp8_pool.tile([128, 8 * btn], F32, tag="rcp")
                nc.vector.reciprocal(rcp, s)
                pfin = tmp32_pool.tile([128, P_HID * btn], F32, tag="pfin")
                nc.vector.memset(
                    pfin.rearrange("p (b k) -> p b k", k=P_HID)[:, :, 32:P_HID],
                    1.0)
                nc.vector.tensor_tensor(
                    pview(pfin, btn),
                    e.rearrange("p (b g c) -> p b g c", b=btn, c=CARD),
                    rcp.rearrange("p (b g) -> p b g", b=btn)
                    .unsqueeze(3).broadcast_to([128, btn, 8, CARD]),
                    op=ALU.mult)
                if SIM_SAFE:
                    # CoreSim's numpy views reject mixed collapsed shapes;
                    # per-bt contiguous calls keep all operands [128, 32]
                    for i in range(btn):
                        nc.vector.copy_predicated(
                            pfin[:, i * P_HID:i * P_HID + 32],
                            m_t[(g, lv)][:, btlo + i, :],
                            oh_t[(g, lv)][:, btlo + i, :])
                else:
                    nc.vector.copy_predicated(
                        pfin.rearrange("p (b k) -> p b k", k=P_HID)[:, :, 0:32],
                        m_t[(g, lv)][:, btlo:btlo + btn, :],
                        oh_t[(g, lv)][:, btlo:btlo + btn, :])
                return pfin

            def osb_store(g, pfin, lv, btlo=0, btn=4):
                dst = (osb_gs[g].rearrange("p (b k) -> p b k", k=OUTW)
                       [:, btlo:btlo + btn, lv * 32:lv * 32 + 32]
                       .rearrange("p b (g c) -> p b g c", c=CARD))
                nc.vector.tensor_copy(dst, pview(pfin, btn))

            def p_transposes(g, pfin, pT_dst, btlo=0, btn=4):
                """pfin [128, btn x (32 probs | 32 ones)] -> pT_dst
                [0:32 probs | ones rows, batch cols] via bf16 PE transpose;
                all btn bt land in one PSUM tile, drained by one copy."""
                pfb = stage_pool.tile([128, P_HID * btn], BF16, tag="pfb")
                nc.vector.tensor_copy(pfb, pfin)
                trp = ps_tr.tile([P_HID, btn, 128], BF16, tag="ptr")
                for i in range(btn):
                    nc.tensor.transpose(
                        trp[:, i, :],
                        pfb[:, i * P_HID:(i + 1) * P_HID], ident_sb)
                base = (4 * g + btlo) * 128
                nc.vector.tensor_copy(
                    pT_dst[:, base:base + btn * 128],
                    trp.rearrange("p b m -> p (b m)"))

            def ps_half(parts=128):
                """[parts, 512] PSUM region from the shared mm pool."""
                if zero_bias:
                    psb = ps_mm.tile([128, 1024], F32, tag="mm", bufs=2)
                    return psb[0:parts, 0:512]
                return ps_mm.tile([parts, 512], F32, tag="mm")

            def mid_h_mms(g):
                # lrelu drain on ACT (Prelu shares the Exp table-set, so no
                # reload); b1c is matmul-folded, no bias needed
                for q in range(4):
                    ps = ps_half()
                    nc.tensor.matmul(
                        ps, w1c_sb[:, q, :],
                        prT_sb[0:P_IN + 1, g * 512:(g + 1) * 512],
                        start=True, stop=True)
                    nc.scalar.activation(
                        hT_sb[:, q, g * 512:(g + 1) * 512], ps,
                        ACT_LRELU, alpha=LRELU_ALPHA)

            def mid_logit_mms(g, ml):
                nc.tensor.matmul(ml, ones_sb, b2cr_sb, start=True, stop=False,
                                 skip_group_check=True)
                for bti in range(4):
                    bt = 4 * g + bti
                    for q in range(4):
                        nc.tensor.matmul(
                            ml[:, bti * 32 + q * 8:bti * 32 + (q + 1) * 8],
                            hT_sb[:, q, bt * 128:(bt + 1) * 128],
                            w2c_sb[:, q, :],
                            start=False, stop=True,
                            skip_group_check=True)

            def task_mms(g, yl, btlo=0, btn=4):
                ps = ps_half(parts=P_HID)
                base = g * 512 + btlo * 128
                nc.tensor.matmul(
                    ps[:, 0:btn * 128], w1y_sb,
                    pmT_sb[0:P_IN + 1, base:base + btn * 128],
                    start=True, stop=True)
                nc.scalar.activation(
                    hyT_sb[0:P_HID, base:base + btn * 128],
                    ps[:, 0:btn * 128], ACT_LRELU, alpha=LRELU_ALPHA)
                for i in range(btn):
                    bt = 4 * g + btlo + i
                    nc.tensor.matmul(
                        yl[:, (btlo + i) * 4:(btlo + i + 1) * 4],
                        hyT_sb[:, bt * 128:(bt + 1) * 128], w2y_sb,
                        start=True, stop=True)

            def y_tail(g, yl, btlo=0, btn=4):
                e4 = tmp8_pool.tile([128, 4 * btn], F32, tag="e4")
                nc.scalar.activation(e4, yl[:, btlo * 4:(btlo + btn) * 4],
                                     AF.Exp)
                s1 = tmp8_pool.tile([128, btn], F32, tag="s1")
                nc.vector.reduce_sum(
                    s1, e4.rearrange("p (b c) -> p b c", c=CARD), axis=AX.X)
                r1 = tmp8_pool.tile([128, btn], F32, tag="r1")
                nc.vector.reciprocal(r1, s1)
                nc.vector.tensor_tensor(
                    osb_gs[g].rearrange("p (b k) -> p b k", k=OUTW)
                    [:, btlo:btlo + btn, 16 * 4:17 * 4],
                    e4.rearrange("p (b c) -> p b c", c=CARD),
                    r1.unsqueeze(2).broadcast_to([128, btn, CARD]),
                    op=ALU.mult)
                # outputs split across the two HWDGE rings to halve the
                # serialized store tail
                for i in range(btn):
                    bti = btlo + i
                    bt = 4 * g + bti
                    eng = nc.sync if bti % 2 == 0 else nc.scalar
                    eng.dma_start(
                        out=out_d.ap()[bt * 128:(bt + 1) * 128, :],
                        in_=osb_gs[g][:, bti * OUTW:(bti + 1) * OUTW])

            # ================= emission schedule =================
            # PE order: transposes bt0-3 (bt-major, chasing the x DMAs) |
            # enc(h0) | vals+scorer(h0) with bt4-7 transposes sprinkled |
            # enc(h1) with h0 root-tail PE interleaved | vals+scorer(h1)
            # with h0 mid/task tail interleaved | tail(h1).
            for bt in range(4):
                transpose_bt(bt)
            encoder_half(0, mid_pe=lambda: transpose_bt(4))

            lg0 = ps_lg.tile([128, 128], F32, tag="lg", name="lg0")
            vals_scorer_half(
                0, lg0,
                extra_pe={2: lambda: transpose_bt(5),
                          4: lambda: transpose_bt(6),
                          6: lambda: transpose_bt(7)})

            # h0 root softmax chain (DVE/ACT) runs under enc(h1) on the PE
            pfin0 = softmax_chain(0, lg0, 0, 1.0 / WSCALE)
            encoder_half(1)
            p_transposes(0, pfin0, prT_sb)
            osb_store(0, pfin0, 0)
            mid_h_mms(0)
            ml0 = ps_lg.tile([128, 128], F32, tag="lg", name="ml0")
            mid_logit_mms(0, ml0)

            lg1 = ps_lg.tile([128, 128], F32, tag="lg", name="lg1")

            def h0_mid_tail():
                pf = softmax_chain(0, ml0, 1, 1.0)
                p_transposes(0, pf, pmT_sb)
                osb_store(0, pf, 1)

            def h0_task():
                yl0 = ps_lg.tile([128, 16], F32, tag="lg", name="yl0")
                task_mms(0, yl0)
                y_tail(0, yl0)

            vals_scorer_half(1, lg1,
                             extra_pe={1: h0_mid_tail, 4: h0_task})

            warm_i = [0]

            def warm(n):
                if zero_bias:
                    ps = ps_mm.tile([128, 1024], F32, tag="mm", bufs=2,
                                    name=f"warm{warm_i[0]}")
                else:
                    ps = ps_mm.tile([128, 512], F32, tag="mm",
                                    name=f"warm{warm_i[0]}")
                warm_i[0] += 1
                for _ in range(n):
                    nc.tensor.matmul(ps[:, 0:128], ident_sb,
                                     ident_sb, start=True, stop=True)

            # ---------------- h1 tail (end of kernel) ----------------
            # pipelined in 2-bt chunks: chunk B's PE transposes run under
            # chunk C's softmax chain on DVE/ACT
            pfB = softmax_chain(1, lg1, 0, 1.0 / WSCALE, 0, 2)
            warm(12)
            pfC = softmax_chain(1, lg1, 0, 1.0 / WSCALE, 2, 2)
            p_transposes(1, pfB, prT_sb, 0, 2)
            osb_store(1, pfB, 0, 0, 2)
            warm(4)
            p_transposes(1, pfC, prT_sb, 2, 2)
            osb_store(1, pfC, 0, 2, 2)
            warm(4)
            mid_h_mms(1)
            ml1 = ps_lg.tile([128, 128], F32, tag="lg", name="ml1")
            mid_logit_mms(1, ml1)
            warm(16)
            pfB2 = softmax_chain(1, ml1, 1, 1.0, 0, 2)
            warm(8)
            pfC2 = softmax_chain(1, ml1, 1, 1.0, 2, 2)
            yl1 = ps_lg.tile([128, 16], F32, tag="lg", name="yl1")
            p_transposes(1, pfB2, pmT_sb, 0, 2)
            osb_store(1, pfB2, 1, 0, 2)
            task_mms(1, yl1, 0, 2)
            p_transposes(1, pfC2, pmT_sb, 2, 2)
            osb_store(1, pfC2, 1, 2, 2)
            task_mms(1, yl1, 2, 2)
            y_tail(1, yl1, 0, 2)
            y_tail(1, yl1, 2, 2)

    nc.compile()
    return nc


def prep_weights(inp):
    """Host-side reformatting of (replicated) weights to device layouts."""
    f8 = ml_dtypes.float8_e4m3
    bf = ml_dtypes.bfloat16
    f32 = np.float32
    W_enc = np.asarray(inp["W_enc"], f32)
    Wv = np.asarray(inp["Wv"], f32)
    Ws = np.asarray(inp["Ws"], f32)
    W1c = np.asarray(inp["W1c"], f32)
    W2c = np.asarray(inp["W2c"], f32)
    W1y = np.asarray(inp["W1y"], f32)
    W2y = np.asarray(inp["W2y"], f32)
    b1c = np.asarray(inp["b1c"], f32)
    b1y = np.asarray(inp["b1y"], f32)
    b2y = np.asarray(inp["b2y"], f32)

    # W2c block-pair layout: [s*64+h, q, s'*4+c] = W2c[2q+s', h, c] iff s==s'
    w2c_bp = np.zeros((2, 64, 4, 2, 4), f32)
    for q in range(4):
        for s in range(2):
            w2c_bp[s, :, q, s, :] = W2c[2 * q + s]  # [h, c]

    # W1c pair layout [32, 4, 2*64] + b1c ones-row -> [33, 512]
    w1c_flat = W1c.transpose(1, 0, 2).reshape(P_IN, 512)
    b1c_row = b1c.reshape(4, 2, 64).reshape(1, 512)
    w1c_aug = np.concatenate([w1c_flat, b1c_row], axis=0)

    w1y_aug = np.concatenate([W1y, b1y.reshape(1, P_HID)], axis=0)
    w2y_aug = np.concatenate([W2y, b2y.reshape(1, CARD)], axis=0)

    wmap = {
        "wenc": np.ascontiguousarray((W_enc * WSCALE).astype(f8)),
        "wv": np.ascontiguousarray(
            (Wv.transpose(1, 0, 2).reshape(D_H, N_ROOT * OV)
             * WSCALE).astype(f8)),
        "ws": np.ascontiguousarray(
            (Ws.transpose(1, 0, 2).reshape(OV, N_ROOT * CARD)
             * WSCALE).astype(f8)),
        "w1c": np.ascontiguousarray(w1c_aug, bf),
        "w2c": np.ascontiguousarray(w2c_bp.reshape(128, 32), bf),
        "w1y": np.ascontiguousarray(w1y_aug, bf),
        "w2y": np.ascontiguousarray(w2y_aug, bf),
        "benc": np.ascontiguousarray(
            np.asarray(inp["b_enc"], f32).reshape(KT_H, 128).T),
        "bv": np.ascontiguousarray(
            np.asarray(inp["bv"], f32).reshape(N_ROOT, 2, 128)
            .transpose(2, 0, 1).reshape(128, 16)),
        "bsr": np.ascontiguousarray(
            np.tile(np.asarray(inp["bs"], f32).reshape(1, 32) * WSCALE,
                    (1, 4)), bf),
        "b2cr": np.ascontiguousarray(
            np.tile(np.asarray(inp["b2c"], f32).reshape(1, 32), (1, 4)), bf),
        "ident": np.ascontiguousarray(np.eye(128), bf),
    }
    return wmap


def make_ohm_host(lab, msk):
    """Device-layout one-hot / mask tensors for one core's shard.

    Returns oh [128, 4*128] f32 and m [128, 4*128] i32 where slot s=2g+lv
    holds [p, bti, grp*4+c] (compact, 32 cols per bt)."""
    oh = np.zeros((128, 4, 4, 32), np.float32)
    m = np.zeros((128, 4, 4, 32), np.int32)
    lab_r = lab.reshape(NBT, 128, 17)   # [bt, p, 17]
    msk_r = msk.reshape(NBT, 128, 17)
    eye = np.eye(CARD, dtype=np.float32)
    for g in range(2):
        for lv in range(2):
            s = 2 * g + lv
            for bti in range(4):
                bt = 4 * g + bti
                l_pg = lab_r[bt, :, lv * 8:lv * 8 + 8]   # [128, 8]
                m_pg = msk_r[bt, :, lv * 8:lv * 8 + 8]
                oh[:, s, bti, :] = eye[l_pg].reshape(128, 32)
                m[:, s, bti, :] = np.repeat(m_pg, CARD, axis=1)
    return (np.ascontiguousarray(oh.reshape(128, 4 * 128)),
            np.ascontiguousarray(m.reshape(128, 4 * 128)))


def make_in_maps(inp):
    wmap = prep_weights(inp)
    x = np.ascontiguousarray(np.asarray(inp["x"], np.float32))
    lab = np.ascontiguousarray(np.asarray(inp["c"], np.int32))
    msk = np.ascontiguousarray(np.asarray(inp["intervention_index"], np.int32))
    in_maps = []
    for i in range(N_CORES):
        m = dict(wmap)
        m["x"] = x[i * BSH:(i + 1) * BSH]
        oh_i, m_i = make_ohm_host(lab[i * BSH:(i + 1) * BSH],
                                  msk[i * BSH:(i + 1) * BSH])
        m["oh"] = oh_i
        m["m"] = m_i
        in_maps.append(m)
    return in_maps


_NC_CACHE = {}


def _get_nc(zero_bias=True):
    key = (SIM_SAFE, NO_DR, zero_bias)
    if key not in _NC_CACHE:
        _NC_CACHE[key] = build_program(zero_bias=zero_bias)
    return _NC_CACHE[key]


def _biases_zero(inputs):
    return all(not np.any(np.asarray(inputs[k]))
               for k in ("b_enc", "bv"))


def kernel(**inputs):
    from concourse.bass_utils import run_bass_kernel_spmd

    nc = _get_nc(zero_bias=_biases_zero(inputs))
    in_maps = make_in_maps(inputs)
    res = run_bass_kernel_spmd(nc, in_maps, list(range(N_CORES)))
    outs = [np.asarray(res.results[i]["out"], np.float32).reshape(BSH, 17, CARD)
            for i in range(N_CORES)]
    return np.concatenate(outs, axis=0)
